# revision 28
# baseline (speedup 1.0000x reference)
"""Multi-head attention (B=2, S=2048, H=1024, 16 heads, RoPE) on 8 trn2 cores.

Sharding: core = (batch b, head-group g); b = core // 4, g = core % 4.
Each core computes 4 heads' attention for one batch and a partial output
projection; the host sums the 4 partials per batch.

v3 vs the 290us f32r baseline (quiet-terminal ~104us):
- Everything bf16 (x/weights/q/k/v/es/aoT/woT/out): a production stream of
  bf16 matmuls runs ~0.5-0.6 cyc/row on TRN2 (2 bf16/cycle moving-operand
  reads) vs 1.0 for f32r -- this is most of the win. fp32 psum throughout.
- Few big DMAs, split across the two HWDGE rings (SP: x blocks + wq + rope
  tables; ACT: wv + wk): DMA *issue* is ~650ns each, serialized per ring,
  and transfers share one pipe, so order = consumption order.
- Phase B order v-proj (streams x blocks as they land) -> k-proj -> q-proj;
  rope emission k-m0, q-m0, k-m1, q-m1 so the first scores item's deps
  clear right as the PE drains the last q matmul.
- All SBUF pools in one scope: phase C es tiles must not be allocated over
  released phase B zones or the allocator's WAR dep stalls the first exp
  behind the last rope op. PSUM pools stay phase-scoped (bank budget).
- kTz zero rows via gpsimd memset (keeps the DMA pipe clear).
- Head-dims within each 128-row m-chunk are permuted to
  [hA d0:32 | hB d0:32 | hA d32:64 | hB d32:64] so RoPE's rotate-half is two
  64-partition DVE ops instead of four 32-partition ones (DVE cost is
  free-size x instruction count; partitions are free). bf16 packed ops get
  the DVE 2x perf mode.
- Compile cache pinned to a fresh dir: stale NEFF-cache hits were observed
  returning a wrong binary for a rebuilt module.
Phase C is ACT-exp-bound on real HW (~4.8us/item) with PE right under it;
per-matmul overhead in a stream is ~2ns (LDWEIGHTS fully hidden), so the
16-matmul/item structure at N=512 is already at the floor.

Attention math: scores are computed in transposed [k, q] layout so attn @ V
needs no transposes; softmax normalization is deferred: V carries a ones
column so the attention matmul also produces the denominator, and gpsimd
partition_broadcast replicates 1/rowsum for the final scale.
"""
import os
import sys
import tempfile

import numpy as np

# The on-disk NEFF cache can serve a stale binary for a rebuilt module with
# an identical signature (observed: silently wrong outputs on cache hits
# after kernel edits). Pin the cache to a fresh per-process dir.
os.environ["NEURON_COMPILE_CACHE_URL"] = tempfile.mkdtemp(prefix="neff-cache-")

sys.path.insert(0, "/opt/trn_rl_repo")

import concourse.bass as bass  # noqa: E402
import concourse.mybir as mybir  # noqa: E402
import concourse.tile as tile  # noqa: E402
from concourse import bacc  # noqa: E402
from concourse.bass_utils import run_bass_kernel_spmd  # noqa: E402

F32 = mybir.dt.float32
F32R = mybir.dt.float32r
BF16 = mybir.dt.bfloat16
EXP = mybir.ActivationFunctionType.Exp

B, S, H = 2, 2048, 1024
NH, D = 16, 64                  # heads, head dim
GH = 4                          # heads per core (group)
GD = GH * D                     # 256 out dims per core
KT = H // 128                   # 8 contraction tiles for projections
MC = S // 128                   # 16 seq chunks of 128
QB = S // 512                   # 4 query blocks of 512
ROPE_BASE = 10000.0
SCALE = D ** -0.5

# Within each 128-row m-chunk of q/k output dims, rows are permuted to
# [hA d0:32 | hB d0:32 | hA d32:64 | hB d32:64] (hA=head 2m, hB=head 2m+1).
# rotate_half then maps rows 64:128 -> 0:64 (factor -sin) and 0:64 -> 64:128
# (factor +sin), each a single 64-partition op. Head hA lives at rows
# {0:32, 64:96}, hB at {32:64, 96:128}.
PERM = np.concatenate([np.arange(0, 32), np.arange(64, 96),
                       np.arange(32, 64), np.arange(96, 128)])


def _rope_tables():
    inv_freq = 1.0 / (ROPE_BASE ** (np.arange(0, D, 2, dtype=np.float64) / D))
    t = np.arange(S, dtype=np.float64)
    freqs = np.outer(t, inv_freq)                     # (S, 32)
    emb = np.concatenate([freqs, freqs], axis=-1)     # (S, 64)
    cosT = np.cos(emb).T                              # (64, S) rows=dim
    sinT = np.sin(emb).T
    # permuted-row tables (d per new row: [0:32, 0:32, 32:64, 32:64])
    cosP = np.concatenate([cosT[0:32], cosT[0:32], cosT[32:64], cosT[32:64]])
    # sinrs at SOURCE rows (equal input base partitions ISA rule):
    #   dest 0:64  <- src 64:128, factor -sin[d_dest]  (rows 64:128 hold it)
    #   dest 64:128 <- src 0:64,  factor +sin[d_dest]  (rows 0:64 hold it)
    # sin[d] == sin[d+32] (emb duplicates freqs), so the factor at a source
    # row equals sin at that row's own d, with the sign of the destination.
    sinP = np.concatenate([sinT[0:32], sinT[0:32], sinT[32:64], sinT[32:64]])
    sinrs = np.concatenate([sinP[0:64], -sinP[64:128]])
    return cosP, sinrs


def _build_nc():
    nc = bacc.Bacc("TRN2", target_bir_lowering=False)
    xT = nc.dram_tensor("xT", [128, QB, KT, 512], BF16, kind="ExternalInput")
    wqT = nc.dram_tensor("wqT", [128, KT, GD], BF16, kind="ExternalInput")
    wkT = nc.dram_tensor("wkT", [128, KT, GD], BF16, kind="ExternalInput")
    wvT = nc.dram_tensor("wvT", [128, KT, GD], BF16, kind="ExternalInput")
    woT = nc.dram_tensor("woT", [128, 2, H], BF16, kind="ExternalInput")
    cos2 = nc.dram_tensor("cos2", [128, S], BF16, kind="ExternalInput")
    sinr = nc.dram_tensor("sinr", [128, S], BF16, kind="ExternalInput")
    outp = nc.dram_tensor("outp", [H, S], BF16, kind="ExternalOutput")

    import os as _os
    _repeat = int(_os.environ.get('KERNEL_REPEAT', '1'))
    with tile.TileContext(nc) as tc:
        with (
            tc.tile_pool(name="const", bufs=1) as const,
            tc.tile_pool(name="persist", bufs=1) as persist,
        ):
            cos_sb = const.tile([128, S], BF16)
            sinr_sb = const.tile([128, S], BF16)

            qT_sb = persist.tile([128, 2, S], BF16)
            # kTz: per-head slots with the other head's rows zeroed, so
            # scores matmuls run at K=128 with a single stationary load
            kTz_sb = persist.tile([128, GH, S], BF16)
            v_sb = persist.tile([128, MC, GH, D + 1], BF16)

            # zero the dead rows of kTz on the idle gpsimd engine (keeps the
            # DMA transfer pipe free for weights/x): head hA (even slot)
            # lives at rows {0:32, 64:96}, hB (odd slot) at {32:64, 96:128}
            nc.gpsimd.memset(kTz_sb[32:64, 0::2, :], 0.0)
            nc.gpsimd.memset(kTz_sb[96:128, 0::2, :], 0.0)
            nc.gpsimd.memset(kTz_sb[0:32, 1::2, :], 0.0)
            nc.gpsimd.memset(kTz_sb[64:96, 1::2, :], 0.0)
            nc.gpsimd.memset(v_sb[:, :, :, D:D + 1], 1.0)

            for _rep in range(_repeat):
                # All SBUF pools share one scope: phase C tiles must not be
                # allocated over released phase B zones, else the allocator's
                # WAR dep makes the first exp wait for the last rope op.
                # PSUM pools stay phase-scoped (8 banks can't hold both).
                with (
                    tc.tile_pool(name="ldw", bufs=1) as ldw,
                    tc.tile_pool(name="pstage", bufs=4) as pstage,
                    tc.tile_pool(name="prot", bufs=2) as prot,
                    tc.tile_pool(name="cpersist", bufs=1) as cpersist,
                    tc.tile_pool(name="es", bufs=2) as es_pool,
                    tc.tile_pool(name="esa3", bufs=3) as esa_pool,
                    tc.tile_pool(name="atmp", bufs=4) as atmp,
                    tc.tile_pool(name="osb", bufs=3) as osb_pool,
                ):
                  # ------------- phase B: projections + rope -------------
                  with (
                    tc.tile_pool(name="ppsum", bufs=4, space="PSUM") as ppsum,
                    tc.tile_pool(name="vpsum", bufs=4, space="PSUM") as vpsum,
                  ):
                    # one DMA per x seq-block + one per weight: DMA issue is
                    # serialized per HWDGE ring at ~650ns each, and transfers
                    # share one ~350GB/s pipe, so order = consumption order.
                    # wk/wv issue on the ACT ring, everything else on SP.
                    xT_sb = ldw.tile([128, QB, KT, 512], BF16)
                    wqT_sb = ldw.tile([128, KT, GD], BF16)
                    wkT_sb = ldw.tile([128, KT, GD], BF16)
                    wvT_sb = ldw.tile([128, KT, GD], BF16)
                    nc.scalar.dma_start(wvT_sb[:], wvT[:])
                    for nb in range(QB):
                        nc.sync.dma_start(xT_sb[:, nb], xT[:, nb])
                    nc.scalar.dma_start(wkT_sb[:], wkT[:])
                    nc.sync.dma_start(wqT_sb[:], wqT[:])
                    if _rep == 0:
                        nc.sync.dma_start(cos_sb[:], cos2[:])
                        nc.sync.dma_start(sinr_sb[:], sinr[:])

                    # v projection first: its psum pool drains early (phase C
                    # psum tiles reuse these banks) and v is ready well before
                    # attn_out(0); consumes x blocks at the DMA streaming rate
                    for mc in range(MC):
                        ps = vpsum.tile([128, GD], F32, tag="vp")
                        for kt in range(KT):
                            nc.tensor.matmul(
                                ps[:],
                                xT_sb[:, mc // 4, kt, bass.ts(mc % 4, 128)],
                                wvT_sb[:, kt, :],
                                start=(kt == 0), stop=(kt == KT - 1),
                            )
                        nc.vector.tensor_copy(
                            v_sb[:, mc, :, 0:D],
                            ps.rearrange("p (h d) -> p h d", h=GH),
                        )

                    # k/q projections + rope. k is nb-outer so each x block
                    # is consumed right as it lands; q (x already resident)
                    # is m-outer. Rope emission order k-m0, q-m0, k-m1, q-m1
                    # so the first scores item's deps clear before the PE
                    # drains the last q matmul.
                    def stage(w_sb, which, m, st):
                        # kt-outer over nb-pairs: the stationary w chunk is
                        # reused across 2 moving blocks, halving weight loads
                        # (x is fully resident once v-proj has streamed it)
                        for pr in range(QB // 2):
                            pss = [ppsum.tile([128, 512], F32, tag="pp",
                                              name=f"pp_{_rep}_{which}_{m}_{pr}_{i}")
                                   for i in range(2)]
                            for kt in range(KT):
                                for i in range(2):
                                    nc.tensor.matmul(
                                        pss[i][:], w_sb[:, kt, bass.ts(m, 128)],
                                        xT_sb[:, 2 * pr + i, kt, :],
                                        start=(kt == 0), stop=(kt == KT - 1),
                                    )
                            for i in range(2):
                                nc.scalar.copy(
                                    st[:, bass.ts(2 * pr + i, 512)], pss[i][:])

                    def rope(which, m, st):
                        tmpR = prot.tile([128, S], BF16, tag="rot",
                                         name=f"tr_{_rep}_{which}_{m}")
                        nc.vector.tensor_mul(tmpR[0:64], st[64:128],
                                             sinr_sb[64:128])
                        nc.vector.tensor_mul(tmpR[64:128], st[0:64],
                                             sinr_sb[0:64])
                        if which == "q":
                            nc.vector.tensor_mul(qT_sb[:, m, :], st[:],
                                                 cos_sb[:])
                            nc.vector.tensor_add(qT_sb[:, m, :],
                                                 qT_sb[:, m, :], tmpR[:])
                        else:
                            tmpC = prot.tile([128, S], BF16, tag="cosp",
                                             name=f"tc_{_rep}_{m}")
                            nc.vector.tensor_mul(tmpC[:], st[:], cos_sb[:])
                            hA, hB = 2 * m, 2 * m + 1
                            for r0, r1, h in ((0, 32, hA), (64, 96, hA),
                                              (32, 64, hB), (96, 128, hB)):
                                nc.vector.tensor_add(
                                    kTz_sb[r0:r1, h, :],
                                    tmpC[r0:r1], tmpR[r0:r1])

                    stk = [pstage.tile([128, S], BF16, tag="stage",
                                       name=f"st_{_rep}_k_{m}")
                           for m in range(2)]
                    stq = [pstage.tile([128, S], BF16, tag="stage",
                                       name=f"st_{_rep}_q_{m}")
                           for m in range(2)]
                    stage(wkT_sb, "k", 0, stk[0])
                    rope("k", 0, stk[0])
                    stage(wkT_sb, "k", 1, stk[1])
                    stage(wqT_sb, "q", 0, stq[0])
                    rope("q", 0, stq[0])
                    rope("k", 1, stk[1])
                    stage(wqT_sb, "q", 1, stq[1])
                    rope("q", 1, stq[1])

                  # -------- phase C: attention + output projection --------
                  # Software-pipelined: iteration i computes scores+exp for
                  # item i and the attn@V / normalize for item i-1.
                  with (
                    tc.tile_pool(name="spsum", bufs=2, space="PSUM") as spsum,
                    tc.tile_pool(name="smallps", bufs=2, space="PSUM") as smallps,
                  ):
                    aoT_sb = cpersist.tile([128, 2, S], BF16)
                    woT_sb = cpersist.tile([128, 2, H], BF16)
                    nc.sync.dma_start(woT_sb[:], woT[:])

                    def scores_exp(qb, h, i):
                        qsl = bass.ts(qb, 512)
                        # two half-item es tiles: ao(i) releases the first half
                        # early so exp(i+2) can start before ao(i) finishes
                        esA = esa_pool.tile([128, MC // 2, 512], BF16, tag="esa",
                                            name=f"esa_{_rep}_{i}")
                        esB = es_pool.tile([128, MC // 2, 512], BF16, tag="esb",
                                           name=f"esb_{_rep}_{i}")
                        kc0 = 0
                        for gsz in (3, 3, 2, 3, 3, 2):
                            es = esA if kc0 < MC // 2 else esB
                            off = 0 if kc0 < MC // 2 else MC // 2
                            sp = spsum.tile([128, 3, 512], F32, tag="sp",
                                            name=f"sp_{_rep}_{i}_{kc0}")
                            for j in range(gsz):
                                kc = kc0 + j
                                nc.tensor.matmul(
                                    sp[:, j, :],
                                    kTz_sb[:, h, bass.ts(kc, 128)],
                                    qT_sb[:, h // 2, qsl],
                                    start=True, stop=True,
                                )
                            nc.scalar.activation(
                                es[:, kc0 - off:kc0 - off + gsz, :],
                                sp[:, 0:gsz, :],
                                EXP, scale=SCALE,
                            )
                            kc0 += gsz
                        return (esA, esB)

                    def attn_out(qb, h, es, i):
                        esA, esB = es
                        qsl = bass.ts(qb, 512)
                        hc, hr = h // 2, (h % 2) * 64
                        ao = smallps.tile([D + 1, 512], F32, tag="ao",
                                          name=f"ao_{_rep}_{i}")
                        for kc in range(MC):
                            eshalf = esA if kc < MC // 2 else esB
                            nc.tensor.matmul(
                                ao[:], v_sb[:, kc, h, :],
                                eshalf[:, kc % (MC // 2), :],
                                start=(kc == 0), stop=(kc == MC - 1),
                            )
                        rcp = atmp.tile([1, 512], F32R, tag="rcp")
                        with nc.allow_low_precision(reason="f32r is fp32-width"):
                            nc.vector.reciprocal(rcp[:], ao[D:D + 1, :])
                        bsb = atmp.tile([D, 512], F32R, tag="bsb")
                        nc.gpsimd.partition_broadcast(bsb[:], rcp[:])
                        nc.vector.tensor_mul(
                            aoT_sb[hr:hr + 64, hc, qsl], ao[0:D, :], bsb[:],
                        )

                    def oproj(qb):
                        # transposed output: partial^T[hid, seq]
                        qsl = bass.ts(qb, 512)
                        for hc8 in range(8):
                            ps = smallps.tile([128, 512], F32, tag="ao",
                                              name=f"op_{_rep}_{qb}_{hc8}")
                            for kt in range(2):
                                nc.tensor.matmul(
                                    ps[:], woT_sb[:, kt, bass.ts(hc8, 128)],
                                    aoT_sb[:, kt, qsl],
                                    start=(kt == 0), stop=(kt == 1),
                                )
                            o_sb = osb_pool.tile([128, 512], BF16, tag="ot")
                            nc.vector.tensor_copy(o_sb[:], ps[:])
                            nc.sync.dma_start(
                                outp[bass.ts(hc8, 128), qsl], o_sb[:],
                            )

                    items = [(qb, h) for qb in range(QB) for h in range(GH)]
                    pending = None
                    for i, (qb, h) in enumerate(items):
                        es = scores_exp(qb, h, i)
                        if pending is not None:
                            pqb, ph, pes, pi = pending
                            attn_out(pqb, ph, pes, pi)
                            if ph == GH - 1:
                                oproj(pqb)
                        pending = (qb, h, es, i)
                    pqb, ph, pes, pi = pending
                    attn_out(pqb, ph, pes, pi)
                    oproj(pqb)

    nc.compile()
    return nc


_NC_CACHE = None
_last_in_maps = None


def _get_nc():
    global _NC_CACHE
    if _NC_CACHE is None:
        _NC_CACHE = _build_nc()
    return _NC_CACHE


def make_in_maps(x, Wq, Wk, Wv, Wo):
    import ml_dtypes
    bf16 = ml_dtypes.bfloat16
    cosP, sinrs = _rope_tables()
    cosP = cosP.astype(bf16)
    sinrs = sinrs.astype(bf16)

    def fold(a):  # [X, F] with X=128*KTI -> [128, KTI, F]
        kti = a.shape[0] // 128
        return np.ascontiguousarray(a.reshape(kti, 128, -1).transpose(1, 0, 2))

    # permuted q/k output-dim order: within each m-chunk of 128, apply PERM
    qk_order = np.concatenate([m * 128 + PERM for m in range(2)])

    in_maps = []
    for core in range(8):
        b, g = core // 4, core % 4
        rows = np.arange(g * GD, (g + 1) * GD)
        xt = fold(np.ascontiguousarray(x[b].T))            # (128, 8, 2048)
        xt = np.ascontiguousarray(
            xt.reshape(128, KT, QB, 512).transpose(0, 2, 1, 3))
        in_maps.append({
            "xT": xt.astype(bf16),                         # (128, 4, 8, 512)
            "wqT": fold(np.ascontiguousarray(Wq[rows[qk_order]].T)).astype(bf16),
            "wkT": fold(np.ascontiguousarray(Wk[rows[qk_order]].T)).astype(bf16),
            "wvT": fold(np.ascontiguousarray(Wv[rows].T)).astype(bf16),
            "woT": fold(np.ascontiguousarray(Wo[:, rows].T)).astype(bf16),
            "cos2": cosP,
            "sinr": sinrs,
        })
    return in_maps


def kernel(x, Wq, Wk, Wv, Wo):
    x = np.asarray(x, dtype=np.float32)
    Wq = np.asarray(Wq, dtype=np.float32)
    Wk = np.asarray(Wk, dtype=np.float32)
    Wv = np.asarray(Wv, dtype=np.float32)
    Wo = np.asarray(Wo, dtype=np.float32)

    global _last_in_maps
    in_maps = make_in_maps(x, Wq, Wk, Wv, Wo)
    _last_in_maps = in_maps
    nc = _get_nc()
    res = run_bass_kernel_spmd(nc, in_maps, core_ids=list(range(8)))
    out = np.zeros((B, S, H), dtype=np.float32)
    for core in range(8):
        out[core // 4] += np.asarray(res.results[core]["outp"],
                                     dtype=np.float32).T
    return out


# revision 29
# speedup vs baseline: 1.4192x; 1.4192x over previous
"""Multi-head attention (B=2, S=2048, H=1024, 16 heads, RoPE) on 8 trn2 cores.

Sharding: core = (batch b, head-group g); b = core // 4, g = core % 4.
Each core computes 4 heads' attention for one batch and a partial output
projection; the host sums the 4 partials per batch.

v3 vs the 290us f32r baseline (quiet-terminal ~104us):
- Everything bf16 (x/weights/q/k/v/es/aoT/woT/out): a production stream of
  bf16 matmuls runs ~0.5-0.6 cyc/row on TRN2 (2 bf16/cycle moving-operand
  reads) vs 1.0 for f32r -- this is most of the win. fp32 psum throughout.
- Few big DMAs, split across the two HWDGE rings (SP: x blocks + wq + rope
  tables; ACT: wv + wk): DMA *issue* is ~650ns each, serialized per ring,
  and transfers share one pipe, so order = consumption order.
- Phase B order v-proj (streams x blocks as they land) -> k-proj -> q-proj;
  rope emission k-m0, q-m0, k-m1, q-m1 so the first scores item's deps
  clear right as the PE drains the last q matmul.
- All SBUF pools in one scope: phase C es tiles must not be allocated over
  released phase B zones or the allocator's WAR dep stalls the first exp
  behind the last rope op. PSUM pools stay phase-scoped (bank budget).
- kTz zero rows via gpsimd memset (keeps the DMA pipe clear).
- Head-dims within each 128-row m-chunk are permuted to
  [hA d0:32 | hB d0:32 | hA d32:64 | hB d32:64] so RoPE's rotate-half is two
  64-partition DVE ops instead of four 32-partition ones (DVE cost is
  free-size x instruction count; partitions are free). bf16 packed ops get
  the DVE 2x perf mode.
- Compile cache pinned to a fresh dir: stale NEFF-cache hits were observed
  returning a wrong binary for a rebuilt module.
Phase C is ACT-exp-bound on real HW (~4.8us/item) with PE right under it;
per-matmul overhead in a stream is ~2ns (LDWEIGHTS fully hidden), so the
16-matmul/item structure at N=512 is already at the floor.

Attention math: scores are computed in transposed [k, q] layout so attn @ V
needs no transposes; softmax normalization is deferred: V carries a ones
column so the attention matmul also produces the denominator, and gpsimd
partition_broadcast replicates 1/rowsum for the final scale.
"""
import os
import sys
import tempfile

import numpy as np

# The on-disk NEFF cache can serve a stale binary for a rebuilt module with
# an identical signature (observed: silently wrong outputs on cache hits
# after kernel edits). Pin the cache to a fresh per-process dir.
os.environ["NEURON_COMPILE_CACHE_URL"] = tempfile.mkdtemp(prefix="neff-cache-")

sys.path.insert(0, "/opt/trn_rl_repo")

import concourse.bass as bass  # noqa: E402
import concourse.mybir as mybir  # noqa: E402
import concourse.tile as tile  # noqa: E402
from concourse import bacc  # noqa: E402
from concourse.bass_utils import run_bass_kernel_spmd  # noqa: E402

F32 = mybir.dt.float32
F32R = mybir.dt.float32r
BF16 = mybir.dt.bfloat16
EXP = mybir.ActivationFunctionType.Exp

B, S, H = 2, 2048, 1024
NH, D = 16, 64                  # heads, head dim
GH = 4                          # heads per core (group)
GD = GH * D                     # 256 out dims per core
KT = H // 128                   # 8 contraction tiles for projections
MC = S // 128                   # 16 seq chunks of 128
QB = S // 512                   # 4 query blocks of 512
ROPE_BASE = 10000.0
SCALE = D ** -0.5

# Within each 128-row m-chunk of q/k output dims, rows are permuted to
# [hA d0:32 | hB d0:32 | hA d32:64 | hB d32:64] (hA=head 2m, hB=head 2m+1).
# rotate_half then maps rows 64:128 -> 0:64 (factor -sin) and 0:64 -> 64:128
# (factor +sin), each a single 64-partition op. Head hA lives at rows
# {0:32, 64:96}, hB at {32:64, 96:128}.
PERM = np.concatenate([np.arange(0, 32), np.arange(64, 96),
                       np.arange(32, 64), np.arange(96, 128)])


def _rope_tables():
    inv_freq = 1.0 / (ROPE_BASE ** (np.arange(0, D, 2, dtype=np.float64) / D))
    t = np.arange(S, dtype=np.float64)
    freqs = np.outer(t, inv_freq)                     # (S, 32)
    emb = np.concatenate([freqs, freqs], axis=-1)     # (S, 64)
    cosT = np.cos(emb).T                              # (64, S) rows=dim
    sinT = np.sin(emb).T
    # permuted-row tables (d per new row: [0:32, 0:32, 32:64, 32:64])
    cosP = np.concatenate([cosT[0:32], cosT[0:32], cosT[32:64], cosT[32:64]])
    # sinrs at SOURCE rows (equal input base partitions ISA rule):
    #   dest 0:64  <- src 64:128, factor -sin[d_dest]  (rows 64:128 hold it)
    #   dest 64:128 <- src 0:64,  factor +sin[d_dest]  (rows 0:64 hold it)
    # sin[d] == sin[d+32] (emb duplicates freqs), so the factor at a source
    # row equals sin at that row's own d, with the sign of the destination.
    sinP = np.concatenate([sinT[0:32], sinT[0:32], sinT[32:64], sinT[32:64]])
    sinrs = np.concatenate([sinP[0:64], -sinP[64:128]])
    return cosP, sinrs


def _build_nc():
    nc = bacc.Bacc("TRN2", target_bir_lowering=False)
    xT = nc.dram_tensor("xT", [128, QB, KT, 512], BF16, kind="ExternalInput")
    wqT = nc.dram_tensor("wqT", [128, KT, GD], BF16, kind="ExternalInput")
    wkT = nc.dram_tensor("wkT", [128, KT, GD], BF16, kind="ExternalInput")
    wvT = nc.dram_tensor("wvT", [128, KT, GD], BF16, kind="ExternalInput")
    woT = nc.dram_tensor("woT", [128, 2, H], BF16, kind="ExternalInput")
    cos2 = nc.dram_tensor("cos2", [128, S], BF16, kind="ExternalInput")
    sinr = nc.dram_tensor("sinr", [128, S], BF16, kind="ExternalInput")
    outp = nc.dram_tensor("outp", [H, S], BF16, kind="ExternalOutput")

    import os as _os
    _repeat = int(_os.environ.get('KERNEL_REPEAT', '1'))
    with tile.TileContext(nc) as tc:
        with (
            tc.tile_pool(name="const", bufs=1) as const,
            tc.tile_pool(name="persist", bufs=1) as persist,
        ):
            cos_sb = const.tile([128, S], BF16)
            sinr_sb = const.tile([128, S], BF16)

            qT_sb = persist.tile([128, 2, S], BF16)
            # kTz: per-head slots with the other head's rows zeroed, so
            # scores matmuls run at K=128 with a single stationary load
            kTz_sb = persist.tile([128, GH, S], BF16)
            v_sb = persist.tile([128, MC, GH, D + 1], BF16)

            # zero the dead rows of kTz on the idle gpsimd engine (keeps the
            # DMA transfer pipe free for weights/x): head hA (even slot)
            # lives at rows {0:32, 64:96}, hB (odd slot) at {32:64, 96:128}
            nc.gpsimd.memset(kTz_sb[32:64, 0::2, :], 0.0)
            nc.gpsimd.memset(kTz_sb[96:128, 0::2, :], 0.0)
            nc.gpsimd.memset(kTz_sb[0:32, 1::2, :], 0.0)
            nc.gpsimd.memset(kTz_sb[64:96, 1::2, :], 0.0)
            nc.gpsimd.memset(v_sb[:, :, :, D:D + 1], 1.0)

            for _rep in range(_repeat):
                # All SBUF pools share one scope: phase C tiles must not be
                # allocated over released phase B zones, else the allocator's
                # WAR dep makes the first exp wait for the last rope op.
                # PSUM pools stay phase-scoped (8 banks can't hold both).
                with (
                    tc.tile_pool(name="ldw", bufs=1) as ldw,
                    tc.tile_pool(name="pstage", bufs=4) as pstage,
                    tc.tile_pool(name="prot", bufs=2) as prot,
                    tc.tile_pool(name="cpersist", bufs=1) as cpersist,
                    tc.tile_pool(name="es", bufs=2) as es_pool,
                    tc.tile_pool(name="esa3", bufs=3) as esa_pool,
                    tc.tile_pool(name="atmp", bufs=4) as atmp,
                    tc.tile_pool(name="osb", bufs=3) as osb_pool,
                ):
                  # ------------- phase B: projections + rope -------------
                  with (
                    tc.tile_pool(name="ppsum", bufs=4, space="PSUM") as ppsum,
                    tc.tile_pool(name="vpsum", bufs=4, space="PSUM") as vpsum,
                  ):
                    # one DMA per x seq-block + one per weight: DMA issue is
                    # serialized per HWDGE ring at ~650ns each, and transfers
                    # share one ~350GB/s pipe, so order = consumption order.
                    # wk/wv issue on the ACT ring, everything else on SP.
                    xT_sb = ldw.tile([128, QB, KT, 512], BF16)
                    wqT_sb = ldw.tile([128, KT, GD], BF16)
                    wkT_sb = ldw.tile([128, KT, GD], BF16)
                    wvT_sb = ldw.tile([128, KT, GD], BF16)
                    nc.scalar.dma_start(wvT_sb[:], wvT[:])
                    for nb in range(QB):
                        nc.sync.dma_start(xT_sb[:, nb], xT[:, nb])
                    nc.scalar.dma_start(wkT_sb[:], wkT[:])
                    nc.sync.dma_start(wqT_sb[:], wqT[:])
                    if _rep == 0:
                        nc.sync.dma_start(cos_sb[:], cos2[:])
                        nc.sync.dma_start(sinr_sb[:], sinr[:])

                    # v projection first: its psum pool drains early (phase C
                    # psum tiles reuse these banks) and v is ready well before
                    # attn_out(0); consumes x blocks at the DMA streaming rate
                    for mc in range(MC):
                        ps = vpsum.tile([128, GD], F32, tag="vp")
                        for kt in range(KT):
                            nc.tensor.matmul(
                                ps[:],
                                xT_sb[:, mc // 4, kt, bass.ts(mc % 4, 128)],
                                wvT_sb[:, kt, :],
                                start=(kt == 0), stop=(kt == KT - 1),
                            )
                        nc.vector.tensor_copy(
                            v_sb[:, mc, :, 0:D],
                            ps.rearrange("p (h d) -> p h d", h=GH),
                        )

                    # k/q projections + rope. k is nb-outer so each x block
                    # is consumed right as it lands; q (x already resident)
                    # is m-outer. Rope emission order k-m0, q-m0, k-m1, q-m1
                    # so the first scores item's deps clear before the PE
                    # drains the last q matmul.
                    def stage(w_sb, which, m, st):
                        # kt-outer over nb-pairs: the stationary w chunk is
                        # reused across 2 moving blocks, halving weight loads
                        # (x is fully resident once v-proj has streamed it)
                        for pr in range(QB // 2):
                            pss = [ppsum.tile([128, 512], F32, tag="pp",
                                              name=f"pp_{_rep}_{which}_{m}_{pr}_{i}")
                                   for i in range(2)]
                            for kt in range(KT):
                                for i in range(2):
                                    nc.tensor.matmul(
                                        pss[i][:], w_sb[:, kt, bass.ts(m, 128)],
                                        xT_sb[:, 2 * pr + i, kt, :],
                                        start=(kt == 0), stop=(kt == KT - 1),
                                    )
                            for i in range(2):
                                nc.scalar.copy(
                                    st[:, bass.ts(2 * pr + i, 512)], pss[i][:])

                    def rope(which, m, st):
                        tmpR = prot.tile([128, S], BF16, tag="rot",
                                         name=f"tr_{_rep}_{which}_{m}")
                        nc.vector.tensor_mul(tmpR[0:64], st[64:128],
                                             sinr_sb[64:128])
                        nc.vector.tensor_mul(tmpR[64:128], st[0:64],
                                             sinr_sb[0:64])
                        if which == "q":
                            nc.vector.tensor_mul(qT_sb[:, m, :], st[:],
                                                 cos_sb[:])
                            nc.vector.tensor_add(qT_sb[:, m, :],
                                                 qT_sb[:, m, :], tmpR[:])
                        else:
                            tmpC = prot.tile([128, S], BF16, tag="cosp",
                                             name=f"tc_{_rep}_{m}")
                            nc.vector.tensor_mul(tmpC[:], st[:], cos_sb[:])
                            hA, hB = 2 * m, 2 * m + 1
                            for r0, r1, h in ((0, 32, hA), (64, 96, hA),
                                              (32, 64, hB), (96, 128, hB)):
                                nc.vector.tensor_add(
                                    kTz_sb[r0:r1, h, :],
                                    tmpC[r0:r1], tmpR[r0:r1])

                    stk = [pstage.tile([128, S], BF16, tag="stage",
                                       name=f"st_{_rep}_k_{m}")
                           for m in range(2)]
                    stq = [pstage.tile([128, S], BF16, tag="stage",
                                       name=f"st_{_rep}_q_{m}")
                           for m in range(2)]
                    stage(wkT_sb, "k", 0, stk[0])
                    rope("k", 0, stk[0])
                    stage(wkT_sb, "k", 1, stk[1])
                    stage(wqT_sb, "q", 0, stq[0])
                    rope("q", 0, stq[0])
                    rope("k", 1, stk[1])
                    stage(wqT_sb, "q", 1, stq[1])
                    rope("q", 1, stq[1])

                  # -------- phase C: attention + output projection --------
                  # Software-pipelined: iteration i computes scores+exp for
                  # item i and the attn@V / normalize for item i-1.
                  with (
                    tc.tile_pool(name="spsum", bufs=2, space="PSUM") as spsum,
                    tc.tile_pool(name="smallps", bufs=2, space="PSUM") as smallps,
                  ):
                    aoT_sb = cpersist.tile([128, 2, S], BF16)
                    woT_sb = cpersist.tile([128, 2, H], BF16)
                    nc.sync.dma_start(woT_sb[:], woT[:])

                    def scores_exp(qb, h, i):
                        qsl = bass.ts(qb, 512)
                        # two half-item es tiles: ao(i) releases the first half
                        # early so exp(i+2) can start before ao(i) finishes
                        esA = esa_pool.tile([128, MC // 2, 512], BF16, tag="esa",
                                            name=f"esa_{_rep}_{i}")
                        esB = es_pool.tile([128, MC // 2, 512], BF16, tag="esb",
                                           name=f"esb_{_rep}_{i}")
                        kc0 = 0
                        for gsz in (3, 3, 2, 3, 3, 2):
                            es = esA if kc0 < MC // 2 else esB
                            off = 0 if kc0 < MC // 2 else MC // 2
                            sp = spsum.tile([128, 3, 512], F32, tag="sp",
                                            name=f"sp_{_rep}_{i}_{kc0}")
                            for j in range(gsz):
                                kc = kc0 + j
                                nc.tensor.matmul(
                                    sp[:, j, :],
                                    kTz_sb[:, h, bass.ts(kc, 128)],
                                    qT_sb[:, h // 2, qsl],
                                    start=True, stop=True,
                                )
                            nc.scalar.activation(
                                es[:, kc0 - off:kc0 - off + gsz, :],
                                sp[:, 0:gsz, :],
                                EXP, scale=SCALE,
                            )
                            kc0 += gsz
                        return (esA, esB)

                    def attn_out(qb, h, es, i):
                        esA, esB = es
                        qsl = bass.ts(qb, 512)
                        hc, hr = h // 2, (h % 2) * 64
                        ao = smallps.tile([D + 1, 512], F32, tag="ao",
                                          name=f"ao_{_rep}_{i}")
                        for kc in range(MC):
                            eshalf = esA if kc < MC // 2 else esB
                            nc.tensor.matmul(
                                ao[:], v_sb[:, kc, h, :],
                                eshalf[:, kc % (MC // 2), :],
                                start=(kc == 0), stop=(kc == MC - 1),
                            )
                        rcp = atmp.tile([1, 512], F32R, tag="rcp")
                        with nc.allow_low_precision(reason="f32r is fp32-width"):
                            nc.vector.reciprocal(rcp[:], ao[D:D + 1, :])
                        bsb = atmp.tile([D, 512], F32R, tag="bsb")
                        nc.gpsimd.partition_broadcast(bsb[:], rcp[:])
                        nc.vector.tensor_mul(
                            aoT_sb[hr:hr + 64, hc, qsl], ao[0:D, :], bsb[:],
                        )

                    def oproj(qb):
                        # transposed output: partial^T[hid, seq]
                        qsl = bass.ts(qb, 512)
                        for hc8 in range(8):
                            ps = smallps.tile([128, 512], F32, tag="ao",
                                              name=f"op_{_rep}_{qb}_{hc8}")
                            for kt in range(2):
                                nc.tensor.matmul(
                                    ps[:], woT_sb[:, kt, bass.ts(hc8, 128)],
                                    aoT_sb[:, kt, qsl],
                                    start=(kt == 0), stop=(kt == 1),
                                )
                            o_sb = osb_pool.tile([128, 512], BF16, tag="ot")
                            nc.vector.tensor_copy(o_sb[:], ps[:])
                            nc.sync.dma_start(
                                outp[bass.ts(hc8, 128), qsl], o_sb[:],
                            )

                    items = [(qb, h) for qb in range(QB) for h in range(GH)]
                    pending = None
                    for i, (qb, h) in enumerate(items):
                        es = scores_exp(qb, h, i)
                        if pending is not None:
                            pqb, ph, pes, pi = pending
                            attn_out(pqb, ph, pes, pi)
                            if ph == GH - 1:
                                oproj(pqb)
                        pending = (qb, h, es, i)
                    pqb, ph, pes, pi = pending
                    attn_out(pqb, ph, pes, pi)
                    oproj(pqb)

    nc.compile()
    return nc


_NC_CACHE = None
_last_in_maps = None


def _get_nc():
    global _NC_CACHE
    if _NC_CACHE is None:
        _NC_CACHE = _build_nc()
    return _NC_CACHE


def make_in_maps(x, Wq, Wk, Wv, Wo):
    import ml_dtypes
    bf16 = ml_dtypes.bfloat16
    cosP, sinrs = _rope_tables()
    cosP = cosP.astype(bf16)
    sinrs = sinrs.astype(bf16)

    def fold(a):  # [X, F] with X=128*KTI -> [128, KTI, F]
        kti = a.shape[0] // 128
        return np.ascontiguousarray(a.reshape(kti, 128, -1).transpose(1, 0, 2))

    # permuted q/k output-dim order: within each m-chunk of 128, apply PERM
    qk_order = np.concatenate([m * 128 + PERM for m in range(2)])

    in_maps = []
    for core in range(8):
        b, g = core // 4, core % 4
        rows = np.arange(g * GD, (g + 1) * GD)
        xt = fold(np.ascontiguousarray(x[b].T))            # (128, 8, 2048)
        xt = np.ascontiguousarray(
            xt.reshape(128, KT, QB, 512).transpose(0, 2, 1, 3))
        in_maps.append({
            "xT": xt.astype(bf16),                         # (128, 4, 8, 512)
            "wqT": fold(np.ascontiguousarray(Wq[rows[qk_order]].T)).astype(bf16),
            "wkT": fold(np.ascontiguousarray(Wk[rows[qk_order]].T)).astype(bf16),
            "wvT": fold(np.ascontiguousarray(Wv[rows].T)).astype(bf16),
            "woT": fold(np.ascontiguousarray(Wo[:, rows].T)).astype(bf16),
            "cos2": cosP,
            "sinr": sinrs,
        })
    return in_maps


def kernel(x, Wq, Wk, Wv, Wo):
    import time as _time

    x = np.asarray(x, dtype=np.float32)
    Wq = np.asarray(Wq, dtype=np.float32)
    Wk = np.asarray(Wk, dtype=np.float32)
    Wv = np.asarray(Wv, dtype=np.float32)
    Wo = np.asarray(Wo, dtype=np.float32)

    global _last_in_maps, _NC_CACHE
    in_maps = make_in_maps(x, Wq, Wk, Wv, Wo)
    _last_in_maps = in_maps
    # the shared axon compile/exec service fails transiently under load;
    # retry with a fresh build + fresh cache dir
    last_exc = None
    for attempt in range(4):
        try:
            nc = _get_nc()
            res = run_bass_kernel_spmd(nc, in_maps, core_ids=list(range(8)))
            break
        except Exception as exc:  # noqa: BLE001
            last_exc = exc
            _NC_CACHE = None
            os.environ["NEURON_COMPILE_CACHE_URL"] = tempfile.mkdtemp(
                prefix="neff-cache-")
            _time.sleep(3.0 * (attempt + 1))
    else:
        raise last_exc
    out = np.zeros((B, S, H), dtype=np.float32)
    for core in range(8):
        out[core // 4] += np.asarray(res.results[core]["outp"],
                                     dtype=np.float32).T
    return out
